# revision 42
# baseline (speedup 1.0000x reference)
"""Fused multi-head attention forward for TRN2, SPMD over 8 NeuronCores.

Problem: B=2, S=2048, D=1024, H=16 heads (Hd=64), fp32.
  out = proj(softmax((x@Wq + bq)(x@Wk + bk)^T / 8) @ (x@Wv + bv))

Sharding: 2-way data parallel over batch x 4-way tensor parallel over heads.
Core c handles batch c//4 and heads [4*(c%4), 4*(c%4)+4). Attention is fully
local; the output projection is computed on each core over its 256 head
features (with bias/4), then a ReduceScatter over each 4-core group sums the
partials and leaves each core with a disjoint 512-row slice of its batch's
output. Host-side work is layout only (slicing, one transpose, concatenation).

On-core layout ("layout B"): the qkv projection produces Q^T/K^T [feat, seq]
so scores are computed transposed (scoresT[k, q] = K^T-stationary x
Q^T-moving); the two heads of a pair occupy PE row-groups (0,0)/(64,0) and
write the two halves of one [128, 1024] PSUM pair-tile, which one ACTIVATE
exponentiates (1/8 scale folded in). The softmax denominator comes free from
a ones-column appended to V in the att@V matmul; the divide is a reciprocal +
K=33 broadcast matmul + DVE multiply. All matmul inputs are float32r
(~1.5e-4 rel err, full PE rate).
"""
import os
import sys

sys.path.insert(0, "/opt/trn_rl_repo")
from contextlib import ExitStack

import numpy as np

import concourse.bass as bass
import concourse.tile as tile
from concourse import bacc, mybir
from concourse.bass_utils import run_bass_kernel_spmd

F32 = mybir.dt.float32
F32R = mybir.dt.float32r
EXP = mybir.ActivationFunctionType.Exp

P = 128
B, S, D, H, HD = 2, 2048, 1024, 16, 64
NH = 4          # heads per core
FQ = NH * HD    # 256 q/k/v features per core
ST = S // P     # 16 seq tiles
KD = D // P     # 8 contraction tiles over d_model
QC = 4          # q chunks
QW = S // QC    # 512
N_CORES = 8
MODE = os.environ.get("MHA_MODE", "rs")  # "rs" (on-device reduce-scatter) or "partial"


def build(mode=MODE):
    nc = bacc.Bacc(
        "TRN2",
        target_bir_lowering=False,
        debug=False,
        enable_asserts=False,
        num_devices=N_CORES,
    )
    xt_d = nc.dram_tensor("xt", [D, S], F32, kind="ExternalInput").ap()
    wqk_d = nc.dram_tensor("wqk", [D, 2 * FQ], F32, kind="ExternalInput").ap()
    wv_d = nc.dram_tensor("wv", [D, FQ], F32, kind="ExternalInput").ap()
    bqk_d = nc.dram_tensor("bqk", [2 * FQ, 1], F32, kind="ExternalInput").ap()
    bv_d = nc.dram_tensor("bv", [1, FQ], F32, kind="ExternalInput").ap()
    wpr_d = nc.dram_tensor("wpr", [FQ, D], F32, kind="ExternalInput").ap()
    bpr_d = nc.dram_tensor("bpr", [1, D], F32, kind="ExternalInput").ap()
    if mode == "rs":
        out_d = nc.dram_tensor("out", [QC, P, D], F32, kind="ExternalOutput").ap()
    else:
        out_d = nc.dram_tensor("out", [S, D], F32, kind="ExternalOutput").ap()

    with tile.TileContext(nc) as tc, ExitStack() as ctx:
        const = ctx.enter_context(tc.tile_pool(name="const", bufs=1))
        qkv = ctx.enter_context(tc.tile_pool(name="qkv", bufs=1))
        otp = ctx.enter_context(tc.tile_pool(name="otp", bufs=1))
        mis = ctx.enter_context(tc.tile_pool(name="mis", bufs=2))
        dpool = ctx.enter_context(tc.tile_pool(name="dram", bufs=1, space="DRAM"))
        pp = ctx.enter_context(tc.tile_pool(name="pp", bufs=2, space="PSUM"))
        # xt / wqk live through phase B (pair-1 qkv is interleaved there)
        xa = ctx.enter_context(tc.tile_pool(name="xt", bufs=1))
        wa = ctx.enter_context(tc.tile_pool(name="wa", bufs=1))

        # ---- constants / small inputs
        wpr_s = [const.tile([P, D], F32R, name=f"wpr{j}") for j in range(2)]
        bpr_s = const.tile([1, D], F32R)
        bv_s = const.tile([1, FQ], F32R)
        bqk_s = []
        for m in range(4):
            t = const.tile([P, 1], F32, name=f"bqk{m}")
            nc.sync.dma_start(t[:], bqk_d[m * P : (m + 1) * P, :])
            bqk_s.append(t)

        sel = const.tile([33, P], F32)
        nc.vector.memset(sel[:], 0.0)
        nc.vector.memset(sel[0:1, 0:64], 1.0)
        nc.vector.memset(sel[32:33, 64:128], 1.0)
        ones_f = const.tile([1, P], F32)
        nc.vector.memset(ones_f[:], 1.0)
        ones128 = const.tile([1, P], F32R)
        nc.vector.tensor_copy(ones128[:], ones_f[:])
        onesv = const.tile([P, ST, NH, 1], F32)
        nc.vector.memset(onesv[:], 1.0)
        bias_bcast = const.tile([P, D], F32)

        # ---- input DMAs (order matters: wqk + xt gate the critical path)
        wqk_s = []
        for k in range(KD):
            t = wa.tile([P, 2 * FQ], F32R, name=f"wqk{k}")
            nc.gpsimd.dma_start(t[:], wqk_d[k * P : (k + 1) * P, :])
            wqk_s.append(t)

        qt_t = [qkv.tile([P, S], F32R, name=f"qt{i}") for i in range(2)]
        kt_t = [qkv.tile([P, S], F32R, name=f"kt{i}") for i in range(2)]
        vt_t = qkv.tile([P, ST, NH, HD + 1], F32R, name="vt")
        nc.vector.tensor_copy(vt_t[:, :, :, HD : HD + 1], onesv[:])

        qk_pending = {}

        def qk_half(m, qc, half):
            # m-tile -> destination: 0,1 = Q pairs; 2,3 = K pairs
            if half == 0:
                qk_pending[(m, qc)] = pp.tile([P, QW], F32, name="pp")
            pq = qk_pending[(m, qc)]
            for k in range(half * KD // 2, (half + 1) * KD // 2):
                nc.tensor.matmul(
                    pq[:],
                    wqk_s[k][:, m * P : (m + 1) * P],
                    xt_s[k][:, qc * QW : (qc + 1) * QW],
                    start=(k == 0),
                    stop=(k == KD - 1),
                )
            if half == 1:
                dest = qt_t[m] if m < 2 else kt_t[m - 2]
                nc.vector.tensor_scalar_add(
                    dest[:, qc * QW : (qc + 1) * QW], pq[:], bqk_s[m][:]
                )
                del qk_pending[(m, qc)]

        def qk_chunk(m, qc):
            qk_half(m, qc, 0)
            qk_half(m, qc, 1)

        with ExitStack() as ctx_v:
            wv_pool = ctx_v.enter_context(tc.tile_pool(name="wv", bufs=1))
            xs = ctx_v.enter_context(tc.tile_pool(name="xs", bufs=3))
            vp = ctx_v.enter_context(tc.tile_pool(name="vp", bufs=2, space="PSUM"))
            wv_s = []
            for k in range(KD):
                t = wv_pool.tile([P, FQ], F32R, name=f"wv{k}")
                nc.gpsimd.dma_start(t[:], wv_d[k * P : (k + 1) * P, :])
                wv_s.append(t)
            nc.gpsimd.dma_start(bv_s[:], bv_d[:])

            # xt via HWDGE + DVE cast (fp32 staging); first halves of every
            # k-tile land before any second half
            xt_s = [xa.tile([P, S], F32R, name=f"x{k}") for k in range(KD)]
            for h in range(2):
                hs = slice(h * S // 2, (h + 1) * S // 2)
                for k in range(KD):
                    stage = xs.tile([P, S // 2], F32, name="stage")
                    nc.sync.dma_start(stage[:], xt_d[k * P : (k + 1) * P, hs])
                    nc.vector.tensor_copy(xt_s[k][:, hs], stage[:])

            for j in range(2):
                nc.gpsimd.dma_start(wpr_s[j][:], wpr_d[j * P : (j + 1) * P, :])
            nc.gpsimd.dma_start(bpr_s[:], bpr_d[:])

            def v_tile(st):
                pv = vp.tile([P, FQ], F32, name="vp")
                for k in range(KD):
                    nc.tensor.matmul(
                        pv[:],
                        xt_s[k][:, st * P : (st + 1) * P],
                        wv_s[k][:],
                        start=(k == 0),
                        stop=False,
                    )
                nc.tensor.matmul(pv[:], ones128[:], bv_s[:], start=False, stop=True)
                nc.vector.tensor_copy(
                    vt_t[:, st, :, 0:HD],
                    pv[:].rearrange("p (a b) -> p a b", a=NH),
                )

            # ---- pair-0 K/Q chunks + V, ordered by data arrival:
            # first-half work, then second-half work
            qk_chunk(2, 0)
            qk_chunk(2, 1)
            qk_chunk(0, 0)
            for st in range(ST // 2):
                v_tile(st)
            qk_chunk(2, 2)
            qk_chunk(2, 3)
            for st in range(ST // 2, ST):
                v_tile(st)

        # ---- phase B: attention pipeline; pair-1 qkv + projection interleaved
        ot_t = [otp.tile([P, S], F32R, name=f"ot{i}") for i in range(2)]
        partial = dpool.tile([S, D], F32, name="partial") if mode == "rs" else out_d

        def proj_block(qc):
            for sub in range(4):
                qt = qc * 4 + sub
                ts = slice(qt * P, (qt + 1) * P)
                outsb = mis.tile([P, D], F32, name="outsb")
                for j in range(2):
                    js = slice(j * QW, (j + 1) * QW)
                    ppp = pp.tile([P, QW], F32, name="pp")
                    nc.tensor.matmul(
                        ppp[:], ot_t[0][:, ts], wpr_s[0][:, js],
                        start=True, stop=False,
                    )
                    nc.tensor.matmul(
                        ppp[:], ot_t[1][:, ts], wpr_s[1][:, js],
                        start=False, stop=True,
                    )
                    nc.vector.tensor_add(outsb[:, js], ppp[:], bias_bcast[:, js])
                nc.sync.dma_start(partial[ts, :], outsb[:])
            if mode == "rs":
                rs_o = dpool.tile([P, D], F32, name=f"rs{qc}")
                nc.gpsimd.collective_compute(
                    "ReduceScatter",
                    mybir.AluOpType.add,
                    replica_groups=[[0, 1, 2, 3], [4, 5, 6, 7]],
                    ins=[partial[qc * QW : (qc + 1) * QW, :].opt()],
                    outs=[rs_o.opt()],
                )
                nc.sync.dma_start(out_d[qc, :, :], rs_o[:])

        with ExitStack() as ctx_b:
            att = ctx_b.enter_context(tc.tile_pool(name="att", bufs=1))
            sp = ctx_b.enter_context(tc.tile_pool(name="sp", bufs=2, space="PSUM"))
            op = ctx_b.enter_context(tc.tile_pool(name="op", bufs=1, space="PSUM"))

            RING = 4
            at = att.tile([P, RING, 2 * QW], F32R, name="at")
            po_cur = {}

            def emit_scores(g, p, qc, kt):
                qs = slice(qc * QW, (qc + 1) * QW)
                ks = slice(kt * P, (kt + 1) * P)
                ps = sp.tile([P, 2 * QW], F32, name="ps")
                nc.tensor.matmul(
                    ps[:, 0:QW], kt_t[p][0:64, ks], qt_t[p][0:64, qs],
                    start=True, stop=True, tile_position=(0, 0),
                )
                nc.tensor.matmul(
                    ps[:, QW : 2 * QW], kt_t[p][64:128, ks], qt_t[p][64:128, qs],
                    start=True, stop=True, tile_position=(64, 0),
                )
                nc.scalar.activation(
                    at[:, g % RING, :], ps[:], EXP, bias=0.0, scale=0.125
                )

            def emit_norm(p, qc):
                qs = slice(qc * QW, (qc + 1) * QW)
                po0, po1 = po_cur.pop((p, qc))
                rab = mis.tile([33, QW], F32, name="rab")
                nc.vector.reciprocal(rab[0:1, :], po0[HD : HD + 1, :])
                nc.vector.reciprocal(rab[32:33, :], po1[HD : HD + 1, :])
                pr = pp.tile([P, QW], F32, name="pp")
                nc.tensor.matmul(pr[:], sel[:], rab[:], start=True, stop=True)
                recipb = mis.tile([P, QW], F32, name="recipb")
                nc.vector.tensor_copy(recipb[:], pr[:])
                nc.vector.tensor_mul(ot_t[p][0:64, qs], po0[0:64, :], recipb[0:64, :])
                nc.vector.tensor_mul(ot_t[p][64:128, qs], po1[0:64, :], recipb[64:128, :])

            def emit_av(g, p, qc, kt):
                if kt == 0:
                    po_cur[(p, qc)] = (
                        op.tile([HD + 1, QW], F32, name="po0"),
                        op.tile([HD + 1, QW], F32, name="po1"),
                    )
                po0, po1 = po_cur[(p, qc)]
                nc.tensor.matmul(
                    po0[:], vt_t[:, kt, 2 * p, :], at[:, g % RING, 0:QW],
                    start=(kt == 0), stop=(kt == ST - 1),
                )
                nc.tensor.matmul(
                    po1[:], vt_t[:, kt, 2 * p + 1, :], at[:, g % RING, QW : 2 * QW],
                    start=(kt == 0), stop=(kt == ST - 1),
                )
                if kt == ST - 1:
                    emit_norm(p, qc)
                    if p == 1:
                        proj_block(qc)

            seq = [
                (p, qc, kt)
                for p in range(2)
                for qc in range(QC)
                for kt in range(ST)
            ]
            DELAY = 3
            # pair-1 Q/K projection chunks, spread through pair-0's rounds
            # remaining projections, interleaved: m0 qc1-3 first (needed by
            # rounds 16/32/48), then K/Q of pair 1 (needed by round 64+)
            early = [(0, qc, h) for qc in (1, 2, 3) for h in range(2)]
            late = [
                (m, qc, h)
                for m in (3, 1)
                for qc in range(QC)
                for h in range(2)
            ]
            m1m3 = {2 + 2 * i: hh for i, hh in enumerate(early)}
            m1m3.update({15 + 3 * i: hh for i, hh in enumerate(late)})
            for g, (p, qc, kt) in enumerate(seq):
                emit_scores(g, p, qc, kt)
                if g == 12:
                    # bias_bcast[p, n] = b_proj[n] (pre-scaled by 1/4 on host)
                    for j in range(2):
                        pb = pp.tile([P, QW], F32, name="pp")
                        nc.tensor.matmul(
                            pb[:], ones128[:], bpr_s[0:1, j * QW : (j + 1) * QW],
                            start=True, stop=True,
                        )
                        nc.vector.tensor_copy(
                            bias_bcast[:, j * QW : (j + 1) * QW], pb[:]
                        )
                if g in m1m3:
                    qk_half(*m1m3[g])
                if g >= DELAY:
                    emit_av(g - DELAY, *seq[g - DELAY])
            for g in range(len(seq) - DELAY, len(seq)):
                emit_av(g, *seq[g])

    nc.compile()
    return nc


_CACHE = {}


def _get_nc(mode=MODE):
    if mode not in _CACHE:
        _CACHE[mode] = build(mode)
    return _CACHE[mode]


def make_in_maps(x, w_qkv, b_qkv, w_proj, b_proj):
    x = np.asarray(x, dtype=np.float32)
    w_qkv = np.asarray(w_qkv, dtype=np.float32)
    b_qkv = np.asarray(b_qkv, dtype=np.float32)
    w_proj = np.asarray(w_proj, dtype=np.float32)
    b_proj = np.asarray(b_proj, dtype=np.float32)
    in_maps = []
    for c in range(N_CORES):
        b, g = c // 4, c % 4
        f = slice(g * FQ, (g + 1) * FQ)
        fq = slice(g * FQ, (g + 1) * FQ)
        fk = slice(D + g * FQ, D + (g + 1) * FQ)
        fv = slice(2 * D + g * FQ, 2 * D + (g + 1) * FQ)
        in_maps.append(
            {
                "xt": np.ascontiguousarray(x[b].T),
                "wqk": np.ascontiguousarray(
                    np.concatenate([w_qkv[:, fq], w_qkv[:, fk]], axis=1)
                ),
                "wv": np.ascontiguousarray(w_qkv[:, fv]),
                "bqk": np.concatenate([b_qkv[fq], b_qkv[fk]]).reshape(2 * FQ, 1).copy(),
                "bv": b_qkv[fv].reshape(1, FQ).copy(),
                "wpr": np.ascontiguousarray(w_proj[f, :]),
                "bpr": (b_proj / 4.0).reshape(1, D).copy(),
            }
        )
    return in_maps


def assemble(results, mode=MODE):
    out = np.empty((B, S, D), dtype=np.float32)
    if mode == "rs":
        for c in range(N_CORES):
            b, i = c // 4, c % 4
            r = results[c]["out"]  # [QC, P, D]
            for qc in range(QC):
                r0 = qc * QW + i * P
                out[b, r0 : r0 + P, :] = r[qc]
    else:
        for b in range(B):
            grp = [results[4 * b + i]["out"] for i in range(4)]
            out[b] = grp[0] + grp[1] + grp[2] + grp[3]
    return out


def kernel(x, w_qkv, b_qkv, w_proj, b_proj, num_heads=H, **_):
    in_maps = make_in_maps(x, w_qkv, b_qkv, w_proj, b_proj)
    try:
        res = run_bass_kernel_spmd(
            _get_nc(MODE), in_maps, core_ids=list(range(N_CORES))
        )
        return assemble(res.results, MODE)
    except Exception:
        if MODE == "partial":
            raise
        # fallback: no-collective program, partial sums reduced on host
        res = run_bass_kernel_spmd(
            _get_nc("partial"), in_maps, core_ids=list(range(N_CORES))
        )
        return assemble(res.results, "partial")


# revision 43
# speedup vs baseline: 1.0327x; 1.0327x over previous
"""Fused multi-head attention forward for TRN2, SPMD over 8 NeuronCores.

Problem: B=2, S=2048, D=1024, H=16 heads (Hd=64), fp32.
  out = proj(softmax((x@Wq + bq)(x@Wk + bk)^T / 8) @ (x@Wv + bv))

Sharding: 2-way data parallel over batch x 4-way tensor parallel over heads.
Core c handles batch c//4 and heads [4*(c%4), 4*(c%4)+4). Attention is fully
local; the output projection is computed on each core over its 256 head
features (with bias/4), then a ReduceScatter over each 4-core group sums the
partials and leaves each core with a disjoint 512-row slice of its batch's
output. Host-side work is layout only (slicing, one transpose, concatenation).

On-core layout ("layout B"): the qkv projection produces Q^T/K^T [feat, seq]
so scores are computed transposed (scoresT[k, q] = K^T-stationary x
Q^T-moving); the two heads of a pair occupy PE row-groups (0,0)/(64,0) and
write the two halves of one [128, 1024] PSUM pair-tile, which one ACTIVATE
exponentiates (1/8 scale folded in). The softmax denominator comes free from
a ones-column appended to V in the att@V matmul; the divide is a reciprocal +
K=33 broadcast matmul + DVE multiply. All matmul inputs are float32r
(~1.5e-4 rel err, full PE rate).
"""
import os
import sys

sys.path.insert(0, "/opt/trn_rl_repo")
from contextlib import ExitStack

import numpy as np

import concourse.bass as bass
import concourse.tile as tile
from concourse import bacc, mybir
from concourse.bass_utils import run_bass_kernel_spmd

F32 = mybir.dt.float32
F32R = mybir.dt.float32r
EXP = mybir.ActivationFunctionType.Exp

P = 128
B, S, D, H, HD = 2, 2048, 1024, 16, 64
NH = 4          # heads per core
FQ = NH * HD    # 256 q/k/v features per core
ST = S // P     # 16 seq tiles
KD = D // P     # 8 contraction tiles over d_model
QC = 4          # q chunks
QW = S // QC    # 512
N_CORES = 8
MODE = os.environ.get("MHA_MODE", "rs")  # "rs" (on-device reduce-scatter) or "partial"


def build(mode=MODE):
    nc = bacc.Bacc(
        "TRN2",
        target_bir_lowering=False,
        debug=False,
        enable_asserts=False,
        num_devices=N_CORES,
    )
    xt_d = nc.dram_tensor("xt", [D, S], F32, kind="ExternalInput").ap()
    wqk_d = nc.dram_tensor("wqk", [D, 2 * FQ], F32, kind="ExternalInput").ap()
    wv_d = nc.dram_tensor("wv", [D, FQ], F32, kind="ExternalInput").ap()
    bqk_d = nc.dram_tensor("bqk", [2 * FQ, 1], F32, kind="ExternalInput").ap()
    bv_d = nc.dram_tensor("bv", [1, FQ], F32, kind="ExternalInput").ap()
    wpr_d = nc.dram_tensor("wpr", [FQ, D], F32, kind="ExternalInput").ap()
    bpr_d = nc.dram_tensor("bpr", [1, D], F32, kind="ExternalInput").ap()
    if mode == "rs":
        out_d = nc.dram_tensor("out", [QC, P, D], F32, kind="ExternalOutput").ap()
    else:
        out_d = nc.dram_tensor("out", [S, D], F32, kind="ExternalOutput").ap()

    with tile.TileContext(nc) as tc, ExitStack() as ctx:
        const = ctx.enter_context(tc.tile_pool(name="const", bufs=1))
        qkv = ctx.enter_context(tc.tile_pool(name="qkv", bufs=1))
        otp = ctx.enter_context(tc.tile_pool(name="otp", bufs=1))
        mis = ctx.enter_context(tc.tile_pool(name="mis", bufs=2))
        dpool = ctx.enter_context(tc.tile_pool(name="dram", bufs=1, space="DRAM"))
        pp = ctx.enter_context(tc.tile_pool(name="pp", bufs=2, space="PSUM"))
        # xt / wqk live through phase B (pair-1 qkv is interleaved there)
        xa = ctx.enter_context(tc.tile_pool(name="xt", bufs=1))
        wa = ctx.enter_context(tc.tile_pool(name="wa", bufs=1))

        # ---- constants / small inputs
        wpr_s = [const.tile([P, D], F32R, name=f"wpr{j}") for j in range(2)]
        bpr_s = const.tile([1, D], F32R)
        bv_s = const.tile([1, FQ], F32R)
        bqk_s = []
        for m in range(4):
            t = const.tile([P, 1], F32, name=f"bqk{m}")
            nc.sync.dma_start(t[:], bqk_d[m * P : (m + 1) * P, :])
            bqk_s.append(t)

        sel_f = const.tile([33, P], F32)
        nc.vector.memset(sel_f[:], 0.0)
        nc.vector.memset(sel_f[0:1, 0:64], 1.0)
        nc.vector.memset(sel_f[32:33, 64:128], 1.0)
        sel = const.tile([33, P], F32R)
        nc.vector.tensor_copy(sel[:], sel_f[:])
        ones_f = const.tile([1, P], F32)
        nc.vector.memset(ones_f[:], 1.0)
        ones128 = const.tile([1, P], F32R)
        nc.vector.tensor_copy(ones128[:], ones_f[:])
        onesv = const.tile([P, ST, NH, 1], F32)
        nc.vector.memset(onesv[:], 1.0)
        bias_bcast = const.tile([P, D], F32)

        # ---- input DMAs (order matters: wqk + xt gate the critical path)
        wqk_s = []
        for k in range(KD):
            t = wa.tile([P, 2 * FQ], F32R, name=f"wqk{k}")
            nc.gpsimd.dma_start(t[:], wqk_d[k * P : (k + 1) * P, :])
            wqk_s.append(t)

        qt_t = [qkv.tile([P, S], F32R, name=f"qt{i}") for i in range(2)]
        kt_t = [qkv.tile([P, S], F32R, name=f"kt{i}") for i in range(2)]
        vt_t = qkv.tile([P, ST, NH, HD + 1], F32R, name="vt")
        nc.vector.tensor_copy(vt_t[:, :, :, HD : HD + 1], onesv[:])

        qk_pending = {}

        def qk_half(m, qc, half):
            # m-tile -> destination: 0,1 = Q pairs; 2,3 = K pairs
            if half == 0:
                qk_pending[(m, qc)] = pp.tile([P, QW], F32, name="pp")
            pq = qk_pending[(m, qc)]
            for k in range(half * KD // 2, (half + 1) * KD // 2):
                nc.tensor.matmul(
                    pq[:],
                    wqk_s[k][:, m * P : (m + 1) * P],
                    xt_s[k][:, qc * QW : (qc + 1) * QW],
                    start=(k == 0),
                    stop=(k == KD - 1),
                )
            if half == 1:
                dest = qt_t[m] if m < 2 else kt_t[m - 2]
                nc.vector.tensor_scalar_add(
                    dest[:, qc * QW : (qc + 1) * QW], pq[:], bqk_s[m][:]
                )
                del qk_pending[(m, qc)]

        def qk_chunk(m, qc):
            qk_half(m, qc, 0)
            qk_half(m, qc, 1)

        with ExitStack() as ctx_v:
            wv_pool = ctx_v.enter_context(tc.tile_pool(name="wv", bufs=1))
            xs = ctx_v.enter_context(tc.tile_pool(name="xs", bufs=3))
            vp = ctx_v.enter_context(tc.tile_pool(name="vp", bufs=2, space="PSUM"))
            wv_s = []
            for k in range(KD):
                t = wv_pool.tile([P, FQ], F32R, name=f"wv{k}")
                nc.gpsimd.dma_start(t[:], wv_d[k * P : (k + 1) * P, :])
                wv_s.append(t)
            nc.gpsimd.dma_start(bv_s[:], bv_d[:])

            # xt via HWDGE + DVE cast (fp32 staging); first halves of every
            # k-tile land before any second half
            xt_s = [xa.tile([P, S], F32R, name=f"x{k}") for k in range(KD)]
            for h in range(2):
                hs = slice(h * S // 2, (h + 1) * S // 2)
                for k in range(KD):
                    stage = xs.tile([P, S // 2], F32, name="stage")
                    nc.sync.dma_start(stage[:], xt_d[k * P : (k + 1) * P, hs])
                    nc.vector.tensor_copy(xt_s[k][:, hs], stage[:])

            for j in range(2):
                nc.gpsimd.dma_start(wpr_s[j][:], wpr_d[j * P : (j + 1) * P, :])
            nc.gpsimd.dma_start(bpr_s[:], bpr_d[:])

            def v_tile(st):
                pv = vp.tile([P, FQ], F32, name="vp")
                for k in range(KD):
                    nc.tensor.matmul(
                        pv[:],
                        xt_s[k][:, st * P : (st + 1) * P],
                        wv_s[k][:],
                        start=(k == 0),
                        stop=False,
                    )
                nc.tensor.matmul(pv[:], ones128[:], bv_s[:], start=False, stop=True)
                nc.vector.tensor_copy(
                    vt_t[:, st, :, 0:HD],
                    pv[:].rearrange("p (a b) -> p a b", a=NH),
                )

            # ---- pair-0 K/Q chunks + V, ordered by data arrival:
            # first-half work, then second-half work
            qk_chunk(2, 0)
            qk_chunk(2, 1)
            qk_chunk(0, 0)
            for st in range(ST // 2):
                v_tile(st)
            qk_chunk(2, 2)
            qk_chunk(2, 3)
            for st in range(ST // 2, ST):
                v_tile(st)

        # ---- phase B: attention pipeline; pair-1 qkv + projection interleaved
        ot_t = [otp.tile([P, S], F32R, name=f"ot{i}") for i in range(2)]
        partial = dpool.tile([S, D], F32, name="partial") if mode == "rs" else out_d

        def proj_block(qc):
            for sub in range(4):
                qt = qc * 4 + sub
                ts = slice(qt * P, (qt + 1) * P)
                outsb = mis.tile([P, D], F32, name="outsb")
                for j in range(2):
                    js = slice(j * QW, (j + 1) * QW)
                    ppp = pp.tile([P, QW], F32, name="pp")
                    nc.tensor.matmul(
                        ppp[:], ot_t[0][:, ts], wpr_s[0][:, js],
                        start=True, stop=False,
                    )
                    nc.tensor.matmul(
                        ppp[:], ot_t[1][:, ts], wpr_s[1][:, js],
                        start=False, stop=True,
                    )
                    nc.vector.tensor_add(outsb[:, js], ppp[:], bias_bcast[:, js])
                nc.sync.dma_start(partial[ts, :], outsb[:])
            if mode == "rs":
                rs_o = dpool.tile([P, D], F32, name=f"rs{qc}")
                nc.gpsimd.collective_compute(
                    "ReduceScatter",
                    mybir.AluOpType.add,
                    replica_groups=[[0, 1, 2, 3], [4, 5, 6, 7]],
                    ins=[partial[qc * QW : (qc + 1) * QW, :].opt()],
                    outs=[rs_o.opt()],
                )
                nc.sync.dma_start(out_d[qc, :, :], rs_o[:])

        with ExitStack() as ctx_b:
            att = ctx_b.enter_context(tc.tile_pool(name="att", bufs=1))
            sp = ctx_b.enter_context(tc.tile_pool(name="sp", bufs=2, space="PSUM"))
            op = ctx_b.enter_context(tc.tile_pool(name="op", bufs=1, space="PSUM"))

            RING = 4
            at = att.tile([P, RING, 2 * QW], F32R, name="at")
            po_cur = {}

            def emit_scores(g, p, qc, kt):
                qs = slice(qc * QW, (qc + 1) * QW)
                ks = slice(kt * P, (kt + 1) * P)
                ps = sp.tile([P, 2 * QW], F32, name="ps")
                nc.tensor.matmul(
                    ps[:, 0:QW], kt_t[p][0:64, ks], qt_t[p][0:64, qs],
                    start=True, stop=True, tile_position=(0, 0),
                )
                nc.tensor.matmul(
                    ps[:, QW : 2 * QW], kt_t[p][64:128, ks], qt_t[p][64:128, qs],
                    start=True, stop=True, tile_position=(64, 0),
                )
                nc.scalar.activation(
                    at[:, g % RING, :], ps[:], EXP, bias=0.0, scale=0.125
                )

            def emit_norm(p, qc):
                qs = slice(qc * QW, (qc + 1) * QW)
                po0, po1 = po_cur.pop((p, qc))
                rab = mis.tile([33, QW], F32R, name="rab")
                with nc.allow_low_precision(reason="f32r softmax recip feeds matmul"):
                    nc.vector.reciprocal(rab[0:1, :], po0[HD : HD + 1, :])
                    nc.vector.reciprocal(rab[32:33, :], po1[HD : HD + 1, :])
                pr = pp.tile([P, QW], F32, name="pp")
                nc.tensor.matmul(pr[:], sel[:], rab[:], start=True, stop=True)
                recipb = mis.tile([P, QW], F32, name="recipb")
                nc.vector.tensor_copy(recipb[:], pr[:])
                nc.vector.tensor_mul(ot_t[p][0:64, qs], po0[0:64, :], recipb[0:64, :])
                nc.vector.tensor_mul(ot_t[p][64:128, qs], po1[0:64, :], recipb[64:128, :])

            def emit_av(g, p, qc, kt):
                if kt == 0:
                    po_cur[(p, qc)] = (
                        op.tile([HD + 1, QW], F32, name="po0"),
                        op.tile([HD + 1, QW], F32, name="po1"),
                    )
                po0, po1 = po_cur[(p, qc)]
                nc.tensor.matmul(
                    po0[:], vt_t[:, kt, 2 * p, :], at[:, g % RING, 0:QW],
                    start=(kt == 0), stop=(kt == ST - 1),
                )
                nc.tensor.matmul(
                    po1[:], vt_t[:, kt, 2 * p + 1, :], at[:, g % RING, QW : 2 * QW],
                    start=(kt == 0), stop=(kt == ST - 1),
                )
                if kt == ST - 1:
                    emit_norm(p, qc)
                    if p == 1:
                        proj_block(qc)

            seq = [
                (p, qc, kt)
                for p in range(2)
                for qc in range(QC)
                for kt in range(ST)
            ]
            DELAY = 3
            # pair-1 Q/K projection chunks, spread through pair-0's rounds
            # remaining projections, interleaved: m0 qc1-3 first (needed by
            # rounds 16/32/48), then K/Q of pair 1 (needed by round 64+)
            early = [(0, qc, h) for qc in (1, 2, 3) for h in range(2)]
            late = [
                (m, qc, h)
                for m in (3, 1)
                for qc in range(QC)
                for h in range(2)
            ]
            m1m3 = {2 + 2 * i: hh for i, hh in enumerate(early)}
            m1m3.update({15 + 3 * i: hh for i, hh in enumerate(late)})
            for g, (p, qc, kt) in enumerate(seq):
                emit_scores(g, p, qc, kt)
                if g == 12:
                    # bias_bcast[p, n] = b_proj[n] (pre-scaled by 1/4 on host)
                    for j in range(2):
                        pb = pp.tile([P, QW], F32, name="pp")
                        nc.tensor.matmul(
                            pb[:], ones128[:], bpr_s[0:1, j * QW : (j + 1) * QW],
                            start=True, stop=True,
                        )
                        nc.vector.tensor_copy(
                            bias_bcast[:, j * QW : (j + 1) * QW], pb[:]
                        )
                if g in m1m3:
                    qk_half(*m1m3[g])
                if g >= DELAY:
                    emit_av(g - DELAY, *seq[g - DELAY])
            for g in range(len(seq) - DELAY, len(seq)):
                emit_av(g, *seq[g])

    nc.compile()
    return nc


_CACHE = {}


def _get_nc(mode=MODE):
    if mode not in _CACHE:
        _CACHE[mode] = build(mode)
    return _CACHE[mode]


def make_in_maps(x, w_qkv, b_qkv, w_proj, b_proj):
    x = np.asarray(x, dtype=np.float32)
    w_qkv = np.asarray(w_qkv, dtype=np.float32)
    b_qkv = np.asarray(b_qkv, dtype=np.float32)
    w_proj = np.asarray(w_proj, dtype=np.float32)
    b_proj = np.asarray(b_proj, dtype=np.float32)
    in_maps = []
    for c in range(N_CORES):
        b, g = c // 4, c % 4
        f = slice(g * FQ, (g + 1) * FQ)
        fq = slice(g * FQ, (g + 1) * FQ)
        fk = slice(D + g * FQ, D + (g + 1) * FQ)
        fv = slice(2 * D + g * FQ, 2 * D + (g + 1) * FQ)
        in_maps.append(
            {
                "xt": np.ascontiguousarray(x[b].T),
                "wqk": np.ascontiguousarray(
                    np.concatenate([w_qkv[:, fq], w_qkv[:, fk]], axis=1)
                ),
                "wv": np.ascontiguousarray(w_qkv[:, fv]),
                "bqk": np.concatenate([b_qkv[fq], b_qkv[fk]]).reshape(2 * FQ, 1).copy(),
                "bv": b_qkv[fv].reshape(1, FQ).copy(),
                "wpr": np.ascontiguousarray(w_proj[f, :]),
                "bpr": (b_proj / 4.0).reshape(1, D).copy(),
            }
        )
    return in_maps


def assemble(results, mode=MODE):
    out = np.empty((B, S, D), dtype=np.float32)
    if mode == "rs":
        for c in range(N_CORES):
            b, i = c // 4, c % 4
            r = results[c]["out"]  # [QC, P, D]
            for qc in range(QC):
                r0 = qc * QW + i * P
                out[b, r0 : r0 + P, :] = r[qc]
    else:
        for b in range(B):
            grp = [results[4 * b + i]["out"] for i in range(4)]
            out[b] = grp[0] + grp[1] + grp[2] + grp[3]
    return out


def kernel(x, w_qkv, b_qkv, w_proj, b_proj, num_heads=H, **_):
    in_maps = make_in_maps(x, w_qkv, b_qkv, w_proj, b_proj)
    try:
        res = run_bass_kernel_spmd(
            _get_nc(MODE), in_maps, core_ids=list(range(N_CORES))
        )
        return assemble(res.results, MODE)
    except Exception:
        if MODE == "partial":
            raise
        # fallback: no-collective program, partial sums reduced on host
        res = run_bass_kernel_spmd(
            _get_nc("partial"), in_maps, core_ids=list(range(N_CORES))
        )
        return assemble(res.results, "partial")
